# revision 12
# baseline (speedup 1.0000x reference)
"""Bias multihead self-attention TRN2 kernel.

Full inputs in, full outputs out. Data-parallel over batch across 8
NeuronCores (2 batches per core). The device kernel works in a fully
"transposed" domain so that no on-device transposes are needed:

  host supplies  xT  = x^T            [B, D, N]
                 wt  = in_proj_w^T    [D, 3D]
                 wot = out_proj_w^T   [D, D]
                 biasT = bias^T(k,q)  [B, H, N, N]
                 m01 = not(pad) f32   [B, 128, 4]   (key 128c+p -> [p, c])

  per core (BL=2 batches):
    qkT[r, n]  = sum_d wt[d, r] * xT[d, n]        (q rows pre-scaled 1/8)
    v[n, j]    = sum_d xT[d, n] * wt[d, 1536+j]   (masked rows zeroed)
    S^T[k, q]  = sum_dh kT[dh, k] * qT[dh, q]     (per head; 2 heads packed
                                                   via PE row groups)
    P^T        = exp(S^T + biasT)                  (ScalarE)
    O^T[dh, q] = sum_k v[k, dh] * P^T[k, q]        (PSUM rows 0..63)
    s[q]       = sum_k m01[k] * P^T[k, q]          (same PSUM bank, row 96,
                                                    PE col group 3)
    ctxT       = O^T * (1/s)  (broadcast via stride-0 SBUF->SBUF DMA)
    y[n, :]    = sum_d ctxT[d, n] * wot[d, :]
"""

import numpy as np
from contextlib import ExitStack

import concourse.bass as bass
import concourse.bacc as bacc
import concourse.tile as tile
import concourse.mybir as mybir
from concourse.bass_utils import run_bass_kernel_spmd

B, N, D, H, DH = 16, 512, 768, 12, 64
NCORES = 8
BL = B // NCORES          # batches per core
DC = D // 128             # 6 contraction chunks
NT = N // 128             # 4 token chunks
QK_TILES = (2 * D) // 128  # 12 row tiles of packed q,k
F32 = mybir.dt.float32
R32 = mybir.dt.float32r
AF = mybir.ActivationFunctionType
ALU = mybir.AluOpType


def _r(ap):
    return ap.bitcast(R32)


def build_kernel(ctx: ExitStack, tc, aps, use_ipb: bool, use_opb: bool):
    nc = tc.nc
    xT, wt, wot, biasT, m01, out = (
        aps["xT"], aps["wt"], aps["wot"], aps["biasT"], aps["m01"], aps["out"],
    )

    # ---- pools ----
    w_pool = ctx.enter_context(tc.tile_pool(name="w", bufs=1))
    xt_pool = ctx.enter_context(tc.tile_pool(name="xt", bufs=1))
    qk_pool = ctx.enter_context(tc.tile_pool(name="qk", bufs=2))
    v_pool = ctx.enter_context(tc.tile_pool(name="v", bufs=2))
    ctx_pool = ctx.enter_context(tc.tile_pool(name="ctx", bufs=1))
    bias_pool = ctx.enter_context(tc.tile_pool(name="bias", bufs=3))
    x_pool = ctx.enter_context(tc.tile_pool(name="xsum", bufs=2))
    pt_pool = ctx.enter_context(tc.tile_pool(name="pt", bufs=2))
    scale_pool = ctx.enter_context(tc.tile_pool(name="scale", bufs=2))
    rp_pool = ctx.enter_context(tc.tile_pool(name="rp", bufs=1))
    y_pool = ctx.enter_context(tc.tile_pool(name="y", bufs=1))

    proj_ps = ctx.enter_context(tc.tile_pool(name="proj_ps", bufs=2, space="PSUM"))
    st_ps = ctx.enter_context(tc.tile_pool(name="st_ps", bufs=3, space="PSUM"))
    av_ps = ctx.enter_context(tc.tile_pool(name="av_ps", bufs=2, space="PSUM"))

    # ---- resident tensors ----
    wt_t = w_pool.tile([128, DC, 3 * D], R32)
    for dc in range(DC):
        nc.sync.dma_start(wt_t[:, dc, :], wt[128 * dc:128 * (dc + 1), :])
    wo_t = w_pool.tile([128, DC, D], R32)
    for dc in range(DC):
        nc.sync.dma_start(wo_t[:, dc, :], wot[128 * dc:128 * (dc + 1), :])
    m01_t = w_pool.tile([128, BL, NT], F32)
    for b in range(BL):
        nc.sync.dma_start(m01_t[:, b, :], m01[b])
    if use_ipb:
        ipb_t = w_pool.tile([128, QK_TILES], F32)
        nc.sync.dma_start(ipb_t[:], aps["ipb"][:])
        ipbv_t = w_pool.tile([128, BL, NT, D], F32)
        for b in range(BL):
            nc.sync.dma_start(ipbv_t[:, b, :, :],
                              aps["ipbv"][b].rearrange("c p j -> p c j"))
    if use_opb:
        opb_t = w_pool.tile([128, D], F32)
        nc.sync.dma_start(opb_t[:], aps["opb"][:])

    # interleave q and k row tiles so head pairs become ready early
    qk_order = []
    for j in range(QK_TILES // 2):
        qk_order += [j, 6 + j]

    for b in range(BL):
        xt_t = xt_pool.tile([128, DC, N], R32, tag="xt")
        for dc in range(DC):
            nc.sync.dma_start(xt_t[:, dc, :], xT[b, 128 * dc:128 * (dc + 1), :])

        # ---- v projection (natural layout), masked rows zeroed at evac ----
        v_t = v_pool.tile([128, NT, H, DH + 1], R32, tag="v")
        for c in range(NT):
            for half in range(2):
                ps = proj_ps.tile([128, 384], F32, tag="proj")
                for dc in range(DC):
                    nc.tensor.matmul(
                        ps[:],
                        (xt_t[:, dc, 128 * c:128 * (c + 1)]),
                        (wt_t[:, dc, 2 * D + 384 * half:2 * D + 384 * (half + 1)]),
                        start=(dc == 0), stop=(dc == DC - 1),
                    )
                vslice = v_t[:, c, 6 * half:6 * (half + 1), 0:DH]
                if use_ipb:
                    nc.vector.scalar_tensor_tensor(
                        vslice, ps[:], m01_t[:, b, c:c + 1],
                        ipbv_t[:, b, c, 384 * half:384 * (half + 1)]
                        .rearrange("p (h e) -> p h e", e=DH),
                        op0=ALU.mult, op1=ALU.add,
                    )
                else:
                    nc.scalar.activation(vslice, ps[:], AF.Copy,
                                         scale=m01_t[:, b, c:c + 1])

        for c in range(NT):
            nc.sync.dma_start(
                v_t[:, c, :, DH:DH + 1],
                aps["m01r"][b, :, c:c + 1]
                .rearrange("p (one c) -> p one c", one=1)
                .broadcast_to([128, H, 1]),
            )

        # ---- q,k projection (transposed layout) ----
        qk_t = qk_pool.tile([128, QK_TILES, N], R32, tag="qk")
        for j in qk_order:
            ps = proj_ps.tile([128, N], F32, tag="proj")
            for dc in range(DC):
                nc.tensor.matmul(
                    ps[:],
                    (wt_t[:, dc, 128 * j:128 * (j + 1)]),
                    (xt_t[:, dc, :]),
                    start=(dc == 0), stop=(dc == DC - 1),
                )
            qscale = 0.125 if j < 6 else 1.0
            if use_ipb:
                nc.scalar.activation(qk_t[:, j, :], ps[:], AF.Identity,
                                     bias=ipb_t[:, j:j + 1], scale=qscale)
            elif j < 6:
                nc.scalar.activation(qk_t[:, j, :], ps[:], AF.Copy, scale=qscale)
            else:
                nc.scalar.copy(qk_t[:, j, :], ps[:])

        # ---- attention, head pair (2j, 2j+1) ----
        for j in range(H // 2):
            av = [av_ps.tile([128, N], F32, tag="av", name=f"av{h2}")
                  for h2 in range(2)]
            for cc in range(2):          # chunk pairs c = 2cc, 2cc+1
                xs = {}
                for h2 in range(2):
                    # scores S^T, two heads packed on PE row groups
                    x_t = x_pool.tile([128, 2, N], F32, tag="xsum")
                    for ci in range(2):
                        c = 2 * cc + ci
                        st = st_ps.tile([128, N], F32, tag="st")
                        nc.tensor.matmul(
                            st[:],
                            (qk_t[64 * h2:64 * (h2 + 1), 6 + j, 128 * c:128 * (c + 1)]),
                            (qk_t[64 * h2:64 * (h2 + 1), j, :]),
                            start=True, stop=True,
                        )
                        bt = bias_pool.tile([128, 2, N], F32, tag="bias")
                        nc.sync.dma_start(
                            bt[:, ci, :],
                            biasT[b, 2 * j + h2, 128 * c:128 * (c + 1), :],
                        )
                        nc.vector.tensor_add(x_t[:, ci, :], st[:], bt[:, ci, :])
                    xs[h2] = x_t
                for h2 in range(2):
                    pt = pt_pool.tile([128, 2, N], R32, tag="pt")
                    nc.scalar.activation(pt[:], xs[h2][:], AF.Exp)
                    for ci in range(2):
                        c = 2 * cc + ci
                        h = 2 * j + h2
                        nc.tensor.matmul(
                            av[h2][0:DH + 1, :],
                            (v_t[:, c, h, 0:DH + 1]),
                            (pt[:, ci, :]),
                            start=(c == 0), stop=(c == NT - 1),
                        )
            # softmax denominator -> broadcast -> normalize into ctxT
            rp = rp_pool.tile([1, 2, N], F32, tag="rp")
            for h2 in range(2):
                nc.vector.reciprocal(rp[0:1, h2, :], av[h2][64:65, :])
            sc = scale_pool.tile([128, N], F32, tag="scale")
            for h2 in range(2):
                nc.scalar.dma_start(
                    sc[64 * h2:64 * (h2 + 1), :],
                    rp[0:1, h2, :].rearrange("p (one q) -> p one q", one=1)
                    .broadcast_to([1, 64, N]),
                )
            if j == 0:
                ctx_t = ctx_pool.tile([128, DC, N], R32, tag="ctx")
            for h2 in range(2):
                nc.vector.tensor_mul(
                    ctx_t[64 * h2:64 * (h2 + 1), j, :],
                    av[h2][0:DH, :],
                    sc[64 * h2:64 * (h2 + 1), :],
                )

        # ---- output projection ----
        for t in range(NT):
            y_t = y_pool.tile([128, D], F32, tag="y")
            for half in range(2):
                ps = proj_ps.tile([128, 384], F32, tag="proj")
                for dc in range(DC):
                    nc.tensor.matmul(
                        ps[:],
                        (ctx_t[:, dc, 128 * t:128 * (t + 1)]),
                        (wo_t[:, dc, 384 * half:384 * (half + 1)]),
                        start=(dc == 0), stop=(dc == DC - 1),
                    )
                yslice = y_t[:, 384 * half:384 * (half + 1)]
                if use_opb:
                    nc.vector.tensor_add(yslice, ps[:],
                                         opb_t[:, 384 * half:384 * (half + 1)])
                else:
                    nc.scalar.copy(yslice, ps[:])
            nc.scalar.dma_start(out[b, 128 * t:128 * (t + 1), :], y_t[:])


def build_program(use_ipb: bool, use_opb: bool):
    nc = bacc.Bacc("TRN2", target_bir_lowering=False, debug=False,
                   num_devices=NCORES)
    aps = {
        "xT": nc.dram_tensor("xT", [BL, D, N], R32, kind="ExternalInput").ap(),
        "wt": nc.dram_tensor("wt", [D, 3 * D], R32, kind="ExternalInput").ap(),
        "wot": nc.dram_tensor("wot", [D, D], R32, kind="ExternalInput").ap(),
        "biasT": nc.dram_tensor("biasT", [BL, H, N, N], F32,
                                kind="ExternalInput").ap(),
        "m01": nc.dram_tensor("m01", [BL, 128, NT], F32,
                              kind="ExternalInput").ap(),
        "m01r": nc.dram_tensor("m01r", [BL, 128, NT], R32,
                               kind="ExternalInput").ap(),
        "out": nc.dram_tensor("out", [BL, N, D], F32, kind="ExternalOutput").ap(),
    }
    if use_ipb:
        aps["ipb"] = nc.dram_tensor("ipb", [128, QK_TILES], F32,
                                    kind="ExternalInput").ap()
        aps["ipbv"] = nc.dram_tensor("ipbv", [BL, NT, 128, D], F32,
                                     kind="ExternalInput").ap()
    if use_opb:
        aps["opb"] = nc.dram_tensor("opb", [128, D], F32,
                                    kind="ExternalInput").ap()
    with tile.TileContext(nc) as tc, ExitStack() as ctx:
        build_kernel(ctx, tc, aps, use_ipb, use_opb)
    nc.compile()
    return nc


_PROGRAMS = {}


def _get_program(use_ipb: bool, use_opb: bool):
    key = (use_ipb, use_opb)
    if key not in _PROGRAMS:
        _PROGRAMS[key] = build_program(use_ipb, use_opb)
    return _PROGRAMS[key]


def make_in_maps(x, key_padding_mask, attn_bias, in_proj_weight, in_proj_bias,
                 out_proj_weight, out_proj_bias, use_ipb, use_opb):
    x = np.asarray(x, np.float32)
    attn_bias = np.asarray(attn_bias, np.float32)
    key_padding_mask = np.asarray(key_padding_mask, bool)
    in_proj_weight = np.asarray(in_proj_weight, np.float32)
    out_proj_weight = np.asarray(out_proj_weight, np.float32)
    in_proj_bias = np.asarray(in_proj_bias, np.float32)
    out_proj_bias = np.asarray(out_proj_bias, np.float32)

    wt_h = np.ascontiguousarray(in_proj_weight.T)
    wot_h = np.ascontiguousarray(out_proj_weight.T)
    xT_h = np.ascontiguousarray(x.transpose(0, 2, 1))
    biasT_h = np.ascontiguousarray(attn_bias.transpose(0, 1, 3, 2))
    m01_h = np.ascontiguousarray(
        (~key_padding_mask).astype(np.float32).reshape(B, NT, 128)
        .transpose(0, 2, 1))

    extra = {}
    if use_ipb:
        ipb_qk = in_proj_bias[:2 * D].reshape(QK_TILES, 128).T.copy()
        ipb_qk[:, :6] *= 0.125  # q bias shares the 1/8 score scale
        extra["ipb"] = np.ascontiguousarray(ipb_qk)
        ipb_v = in_proj_bias[2 * D:]
        # ipbv[b, c, p, j] = ipb_v[j] * m01[b, 128c+p]
        m01_flat = (~key_padding_mask).astype(np.float32).reshape(B, NT, 128)
        extra["ipbv"] = np.ascontiguousarray(
            m01_flat[:, :, :, None] * ipb_v[None, None, None, :])
    if use_opb:
        extra["opb"] = np.ascontiguousarray(
            np.broadcast_to(out_proj_bias[None, :], (128, D)))

    in_maps = []
    for c0 in range(NCORES):
        sl = slice(BL * c0, BL * (c0 + 1))
        m = {
            "xT": xT_h[sl],
            "wt": wt_h,
            "wot": wot_h,
            "biasT": biasT_h[sl],
            "m01": m01_h[sl],
            "m01r": m01_h[sl],
        }
        if use_ipb:
            m["ipb"] = extra["ipb"]
            m["ipbv"] = extra["ipbv"][sl]
        if use_opb:
            m["opb"] = extra["opb"]
        in_maps.append(m)
    return in_maps


def kernel(x, key_padding_mask, attn_bias, in_proj_weight, in_proj_bias,
           out_proj_weight, out_proj_bias, _spmd_kwargs=None):
    use_ipb = bool(np.any(np.asarray(in_proj_bias)))
    use_opb = bool(np.any(np.asarray(out_proj_bias)))
    nc = _get_program(use_ipb, use_opb)
    in_maps = make_in_maps(x, key_padding_mask, attn_bias, in_proj_weight,
                           in_proj_bias, out_proj_weight, out_proj_bias,
                           use_ipb, use_opb)
    res = run_bass_kernel_spmd(nc, in_maps, core_ids=list(range(NCORES)),
                               **(_spmd_kwargs or {}))
    out = np.concatenate([res.results[c]["out"] for c in range(NCORES)], axis=0)
    if _spmd_kwargs:
        kernel.last_results = res
    return np.ascontiguousarray(out.astype(np.float32))
